# revision 26
# baseline (speedup 1.0000x reference)
"""Fused pre-LN transformer block (causal MHA + FFN) on 8 TRN2 NeuronCores.

Sharding: core c handles batch b = c//2 and head-half hh = c%2 (8 of 16 heads).
Attention runs fully local per (batch, head-half); the attention projection
produces a partial sum that is ReduceScattered (over token dim) within each
core pair, so FFN runs token-sharded (1024 tokens/core). Output per core is
its token slice, stored E-major [E, TH]; the host transposes when gathering.

v4: per-engine streams are executed in emission order, so the program is
emitted as one interleaved schedule:
- LN1+QKV software-pipelined over 4 token chunks.
- Attention emitted column-outer (q-chunk) x head with exp groups of 3
  k-blocks; the psO (attn-V) matmuls for group g are emitted after the
  scores of group g+1, so the PE never sits behind the scalar engine's exp.
- Softmax denominators staged to 8 partitions; one batched reciprocal per
  4 heads; normalization applied lazily in-place on the raw attn output.
- The attention projection for column c is emitted in 2-E-tile slices
  between the heads of column c+1 (fills the column-boundary bubble);
  the two bf16 pair ReduceScatters are issued mid-column-3 / right after.
- LN2 for token-half A is emitted inside the attention region (overlaps
  proj(3)/RS_B); LN2 for half B is woven into the middle of ff1(A).
- FFN: fw1 streamed in 1MB eighths per half, fw2 in quarters; relu and
  biases fused on the scalar engine.
"""

import numpy as np
import ml_dtypes

import concourse.bass as bass
import concourse.mybir as mybir
import concourse.tile as tile
from concourse import bacc
from concourse.bass import ts, ds
from concourse.bass_utils import run_bass_kernel_spmd

BF16 = mybir.dt.bfloat16
F32 = mybir.dt.float32
NPBF16 = ml_dtypes.bfloat16

B, T, E = 4, 2048, 1024
H, HS = 16, 64
FF = 4 * E
EPS = 1e-5
NCORES = 8
HPC = 8            # heads per core
HD = HPC * HS      # 512 head dims per core
TH = T // 2        # 1024 tokens per core for FFN
KT_N = T // 128    # 16 k-tiles
ET = E // 128      # 8 e-tiles
ADD = mybir.AluOpType.add
SUB = mybir.AluOpType.subtract
MUL = mybir.AluOpType.mult
EXP = mybir.ActivationFunctionType.Exp
RELU = mybir.ActivationFunctionType.Relu
SQRT = mybir.ActivationFunctionType.Sqrt
COPY = mybir.ActivationFunctionType.Copy
IDENT = mybir.ActivationFunctionType.Identity


def build_program(single=False, dbg=False):
    nc = bacc.Bacc("TRN2", target_bir_lowering=False, debug=False,
                   num_devices=1 if single else NCORES)
    dbg_t = {}
    if dbg:
        for name, shape, dt in (
                ("dQT", [128, 4, T], BF16), ("dKT", [128, 4, T], BF16),
                ("dVp", [128, KT_N, HPC, 65], BF16),
                ("dAO", [128, 4, T], BF16), ("dLA", [97, 4, 512], F32),
                ("dLB", [97, 4, 512], F32), ("drsA", [E, 512], BF16),
                ("drsB", [E, 512], BF16), ("dxp", [128, ET, TH], F32)):
            dbg_t[name] = nc.dram_tensor(name, shape, dt,
                                         kind="ExternalOutput").ap()

    # ---- I/O ----
    xT = nc.dram_tensor("xT", [E, T], BF16, kind="ExternalInput").ap()
    xrpT = nc.dram_tensor("xrpT", [E, TH], BF16, kind="ExternalInput").ap()
    qw = nc.dram_tensor("qw", [E, HD], BF16, kind="ExternalInput").ap()
    kw = nc.dram_tensor("kw", [E, HD], BF16, kind="ExternalInput").ap()
    vw = nc.dram_tensor("vw", [E, HD], BF16, kind="ExternalInput").ap()
    apw = nc.dram_tensor("apw", [HD, E], BF16, kind="ExternalInput").ap()
    fw1 = nc.dram_tensor("fw1", [E, FF], BF16, kind="ExternalInput").ap()
    fb1 = nc.dram_tensor("fb1", [128, FF // 128], F32,
                         kind="ExternalInput").ap()
    fw2 = nc.dram_tensor("fw2", [FF, E], BF16, kind="ExternalInput").ap()
    fb2 = nc.dram_tensor("fb2", [128, ET], F32, kind="ExternalInput").ap()
    out = nc.dram_tensor("out", [E, TH], F32, kind="ExternalOutput").ap()

    # internal DRAM for the two chunked pair-wise reduce-scatters (bf16).
    # partA holds global token quarters {0, 2}, partB {1, 3}; RS over the
    # core pair scatters dim0, so the even core gets quarters 0,1 and the
    # odd core quarters 2,3 == its own token half.
    partA = nc.dram_tensor("partA", [2, E, 512], BF16).ap()
    partB = nc.dram_tensor("partB", [2, E, 512], BF16).ap()
    rsA = nc.dram_tensor("rsA", [E, 512], BF16).ap()
    rsB = nc.dram_tensor("rsB", [E, 512], BF16).ap()
    groups = [[0, 1], [2, 3], [4, 5], [6, 7]]

    def _emit(tc):
        with tc.tile_pool(name="const", bufs=1) as constp:
            ones_bf = constp.tile([128, 1], BF16)
            nc.gpsimd.memset(ones_bf, 1.0)
            eps_sb = constp.tile([1, 1], F32)
            nc.gpsimd.memset(eps_sb, EPS)
            scratch1 = constp.tile([1, 64], F32)
            nc.gpsimd.memset(scratch1, 1.0)
            fb1_sb = constp.tile([128, FF // 128], F32)
            fb2_sb = constp.tile([128, ET], F32)

            # x' (post-attention residual input), lives to the end.
            xp = constp.tile([128, ET, TH], F32, name="xp")
            # LN2 output for token half A (computed in the attn region)
            h2A = constp.tile([128, ET, 512], BF16, name="h2A")

            def emit_ln2_pre(half, pool, on_act=True):
                hsl = ds(512 * half, 512)
                xpb = pool.tile([128, ET, 512], BF16, tag=f"xpb{half}",
                                bufs=1)
                if on_act:
                    nc.scalar.activation(xpb, xp[:, :, hsl], COPY)
                else:
                    nc.vector.tensor_copy(xpb, xp[:, :, hsl])
                xqb = pool.tile([128, ET, 512], BF16, tag=f"xqb{half}",
                                bufs=1)
                nc.vector.tensor_tensor(xqb, xpb, xpb, MUL)
                return xpb, xqb

            def emit_ln2_post(half, pool, psp, pstag, xpb, xqb, dst):
                psm = psp.tile([1, 512], F32, tag=pstag, bufs=2)
                psq = psp.tile([1, 512], F32, tag=pstag, bufs=2)
                for i in range(ET):
                    nc.tensor.matmul(psm, ones_bf, xpb[:, i, :],
                                     start=(i == 0), stop=(i == 7))
                for i in range(ET):
                    nc.tensor.matmul(psq, ones_bf, xqb[:, i, :],
                                     start=(i == 0), stop=(i == 7))
                mu = pool.tile([1, 512], F32, tag=f"mu_{half}", bufs=1)
                rstd = pool.tile([1, 512], F32, tag=f"rstd_{half}", bufs=1)
                msr = pool.tile([1, 512], F32, tag=f"msr_{half}", bufs=1)
                nc.vector.tensor_scalar_mul(mu, psm, 1.0 / E)
                nc.vector.tensor_scalar_mul(rstd, psq, 1.0 / E)
                nc.vector.tensor_tensor(msr, mu, mu, MUL)
                nc.vector.tensor_tensor(rstd, rstd, msr, SUB)
                nc.scalar.activation(rstd, rstd, SQRT, bias=eps_sb)
                nc.vector.reciprocal(rstd, rstd)
                nc.vector.tensor_tensor(msr, mu, rstd, MUL)
                bc_rs = pool.tile([128, 512], F32, tag=f"bcr_{half}",
                                  bufs=1)
                nc.gpsimd.partition_broadcast(bc_rs, rstd)
                bc_ms = pool.tile([128, 512], F32, tag=f"bcm_{half}",
                                  bufs=1)
                nc.gpsimd.partition_broadcast(bc_ms, msr)
                nc.vector.tensor_tensor(
                    dst, xpb,
                    bc_rs.unsqueeze(1).broadcast_to([128, ET, 512]), MUL)
                nc.vector.tensor_tensor(
                    dst, dst,
                    bc_ms.unsqueeze(1).broadcast_to([128, ET, 512]), SUB)

            def emit_res_half(half, rsx, pool, xsrc):
                hsl = ds(512 * half, 512)
                rsb = pool.tile([128, ET, 512], BF16, tag=f"rsb{half}",
                                bufs=1)
                nc.sync.dma_start(
                    out=rsb, in_=rsx.rearrange("(j p) t -> p j t", p=128))
                nc.vector.tensor_tensor(xp[:, :, hsl], rsb, xsrc, ADD)

            # ======== phase A: attention ========
            with tc.tile_pool(name="persA", bufs=1) as pA:
                QT = pA.tile([128, 4, T], BF16, name="QT")
                KT = pA.tile([128, 4, T], BF16, name="KT")
                AO = pA.tile([128, 4, T], BF16, name="AO")
                Vp = pA.tile([128, KT_N, HPC, 65], BF16, name="Vp")
                nc.vector.memset(Vp[:, :, :, 64:65], 1.0)
                # softmax denominators: 4 heads per tile on partitions
                # {0,32,64,96} (DVE partition-start constraint), so the
                # per-column reciprocal batches 4 heads per op.
                LstA = pA.tile([97, 4, 512], F32, name="LstA")
                LstB = pA.tile([97, 4, 512], F32, name="LstB")
                nc.vector.memset(LstA, 1.0)
                nc.vector.memset(LstB, 1.0)
                # causal masks for the 4 diagonal k-offsets:
                # masks[p, kk, qq] = 1 if qq >= 128*kk + p else 0
                masks = pA.tile([128, 4, 512], BF16, name="masks")
                nc.gpsimd.memset(masks, 1.0)
                nc.gpsimd.affine_select(
                    out=masks, in_=masks, compare_op=mybir.AluOpType.is_ge,
                    fill=0.0, base=0, pattern=[[-128, 4], [1, 512]],
                    channel_multiplier=-1)

                # ---- A1: LN1 + QKV, software-pipelined over 4 chunks ----
                with tc.tile_pool(name="ln1", bufs=1) as sb, \
                     tc.tile_pool(name="ln1_ps", bufs=1, space="PSUM") as ps:
                    xTs = sb.tile([128, ET, T], BF16, name="xTs")
                    w_q = sb.tile([128, ET, HD], BF16, name="w_q")
                    w_k = sb.tile([128, ET, HD], BF16, name="w_k")
                    w_v = sb.tile([128, ET, HD], BF16, name="w_v")
                    nc.sync.dma_start(
                        out=xTs[:, :, ds(0, 512)],
                        in_=xT.rearrange("(i p) t -> p i t",
                                         p=128)[:, :, ds(0, 512)])
                    nc.sync.dma_start(
                        out=w_q, in_=qw.rearrange("(i p) f -> p i f", p=128))
                    nc.sync.dma_start(
                        out=w_k, in_=kw.rearrange("(i p) f -> p i f", p=128))
                    nc.sync.dma_start(
                        out=w_v, in_=vw.rearrange("(i p) f -> p i f", p=128))

                    def emit_stats(c):
                        csl = ds(512 * c, 512)
                        if c > 0:
                            nc.sync.dma_start(
                                out=xTs[:, :, csl],
                                in_=xT.rearrange("(i p) t -> p i t",
                                                 p=128)[:, :, csl])
                        xsq = sb.tile([128, ET, 512], BF16, tag="xsq",
                                      bufs=1)
                        nc.vector.tensor_tensor(
                            xsq, xTs[:, :, csl], xTs[:, :, csl], MUL)
                        psm = ps.tile([1, 512], F32, tag="psm", bufs=3)
                        psq = ps.tile([1, 512], F32, tag="psq", bufs=3)
                        for i in range(ET):
                            nc.tensor.matmul(
                                psm, ones_bf, xTs[:, i, csl],
                                start=(i == 0), stop=(i == 7))
                        for i in range(ET):
                            nc.tensor.matmul(
                                psq, ones_bf, xsq[:, i, :],
                                start=(i == 0), stop=(i == 7))
                        mu = sb.tile([1, 512], F32, tag="mu", bufs=1)
                        rstd = sb.tile([1, 512], F32, tag="rstd", bufs=1)
                        msr = sb.tile([1, 512], F32, tag="msr", bufs=1)
                        nc.vector.tensor_scalar_mul(mu, psm, 1.0 / E)
                        nc.vector.tensor_scalar_mul(rstd, psq, 1.0 / E)
                        nc.vector.tensor_tensor(msr, mu, mu, MUL)
                        nc.vector.tensor_tensor(rstd, rstd, msr, SUB)
                        # rstd = 1/sqrt(var + eps)
                        nc.scalar.activation(rstd, rstd, SQRT, bias=eps_sb)
                        nc.vector.reciprocal(rstd, rstd)
                        nc.vector.tensor_tensor(msr, mu, rstd, MUL)
                        bc_rs = sb.tile([128, 512], F32, tag="bc_rs", bufs=2)
                        nc.gpsimd.partition_broadcast(bc_rs, rstd)
                        bc_ms = sb.tile([128, 512], F32, tag="bc_ms", bufs=2)
                        nc.gpsimd.partition_broadcast(bc_ms, msr)
                        # normalize in place: h = x*rstd - mu*rstd
                        nc.vector.tensor_tensor(
                            xTs[:, :, csl], xTs[:, :, csl],
                            bc_rs.unsqueeze(1).broadcast_to([128, ET, 512]),
                            MUL)
                        nc.vector.tensor_tensor(
                            xTs[:, :, csl], xTs[:, :, csl],
                            bc_ms.unsqueeze(1).broadcast_to([128, ET, 512]),
                            SUB)

                    def emit_qkv(c):
                        csl = ds(512 * c, 512)
                        for wi, (w_sb, o_sb) in enumerate(
                                ((w_q, QT), (w_k, KT))):
                            for m in range(4):
                                pq = ps.tile([128, 512], F32, tag="pqv",
                                             bufs=2)
                                for i in range(ET):
                                    nc.tensor.matmul(
                                        pq, w_sb[:, i, ts(m, 128)],
                                        xTs[:, i, csl],
                                        start=(i == 0), stop=(i == 7))
                                if (wi * 4 + m) % 2 == 0:
                                    nc.vector.tensor_copy(
                                        o_sb[:, m, csl], pq)
                                else:
                                    nc.scalar.activation(
                                        o_sb[:, m, csl], pq, COPY)
                        for mt in range(4):
                            kt = 4 * c + mt
                            pv = ps.tile([128, 512], F32, tag="pqv", bufs=2)
                            for i in range(ET):
                                nc.tensor.matmul(
                                    pv, xTs[:, i, ts(kt, 128)], w_v[:, i, :],
                                    start=(i == 0), stop=(i == 7))
                            vdst = Vp[:, kt, :, 0:64]
                            pvr = pv.rearrange("p (h d) -> p h d", h=HPC)
                            if mt % 2 == 0:
                                nc.vector.tensor_copy(vdst, pvr)
                            else:
                                nc.scalar.activation(vdst, pvr, COPY)

                    emit_stats(0)
                    emit_stats(1)
                    emit_stats(2)
                    emit_qkv(0)
                    emit_stats(3)
                    emit_qkv(1)
                    emit_qkv(2)
                    emit_qkv(3)
                    # preload the exp table while QKV finishes
                    nc.scalar.activation(scratch1, scratch1, EXP)

                # ---- A2+A3: attention/proj/RS/LN2-A interleaved ----
                with tc.tile_pool(name="att", bufs=1) as sb, \
                     tc.tile_pool(name="att_ps", bufs=1,
                                  space="PSUM") as ps:
                    apws = sb.tile([128, 4, E], BF16, name="apws")
                    nc.sync.dma_start(
                        out=apws, in_=apw.rearrange("(k p) e -> p k e",
                                                    p=128))
                    xrs = sb.tile([128, ET, 512], BF16, name="xrs")
                    nc.sync.dma_start(
                        out=xrs,
                        in_=xrpT.rearrange("(i p) t -> p i t",
                                           p=128)[:, :, ds(0, 512)])
                    G = 3

                    def emit_norm(c, heads, rcl):
                        csl = ts(c, 512)
                        for h in heads:
                            hp, z = h // 2, h % 2
                            pp = slice(64 * z, 64 * z + 64)
                            lp = 32 * (h % 4)
                            # HW partition_broadcast reads the tile's
                            # partition 0 only -> bounce through a p0 row.
                            t0 = sb.tile([1, 512], F32, tag="t0", bufs=2)
                            nc.vector.tensor_copy(t0, rcl[lp:lp + 1, :])
                            rbc = sb.tile([128, 512], F32, tag="rbc",
                                          bufs=2)
                            nc.gpsimd.partition_broadcast(rbc, t0)
                            nc.vector.tensor_tensor(
                                AO[pp, hp, csl], AO[pp, hp, csl],
                                rbc[pp, :], MUL)

                    def emit_head(c, h):
                        csl = ts(c, 512)
                        njs = 4 * (c + 1)
                        hp, z = h // 2, h % 2
                        pp = slice(64 * z, 64 * z + 64)
                        psO = ps.tile([65, 512], F32, tag="psO", bufs=2,
                                      name="psO")

                        def psO_mm(js, PT):
                            for idx, j in enumerate(js):
                                nc.tensor.matmul(
                                    psO, Vp[:, j, h, :], PT[:, idx, :],
                                    start=(j == 0), stop=(j == njs - 1))

                        pend = None
                        for g0 in range(0, njs, G):
                            js = list(range(g0, min(g0 + G, njs)))
                            n = len(js)
                            pS = ps.tile([128, G, 512], F32, tag="pS",
                                         bufs=2, name="pS")
                            for idx, j in enumerate(js):
                                nc.tensor.matmul(
                                    pS[:, idx, :], KT[pp, hp, ts(j, 128)],
                                    QT[pp, hp, csl], start=True, stop=True)
                            PT = sb.tile([128, G, 512], BF16, tag="PT",
                                         bufs=2, name="PT")
                            nc.scalar.activation(
                                PT[:, 0:n, :], pS[:, 0:n, :], EXP,
                                scale=float(HS) ** -0.5)
                            d0 = 4 * c
                            if js[-1] >= d0:
                                lo = max(js[0], d0)
                                a = lo - js[0]
                                nc.vector.tensor_tensor(
                                    PT[:, a:n, :], PT[:, a:n, :],
                                    masks[:, lo - d0:js[-1] - d0 + 1, :],
                                    MUL)
                            if pend is not None:
                                psO_mm(*pend)
                            pend = (js, PT)
                        psO_mm(*pend)
                        # stage raw output + denominator; normalize later
                        nc.vector.tensor_copy(AO[pp, hp, csl], psO[0:64, :])
                        lst = LstA if h < 4 else LstB
                        lp = 32 * (h % 4)
                        nc.vector.tensor_copy(lst[lp:lp + 1, c, :],
                                              psO[64:65, :])
                        if h == 3:
                            # heads 0-3 normalize overlaps heads 4-7
                            rclA = sb.tile([97, 512], F32, tag="rclA",
                                           bufs=1)
                            nc.vector.reciprocal(rclA, LstA[:, c, :])
                            emit_norm(c, range(0, 4), rclA)
                        if h == 7:
                            rclB = sb.tile([97, 512], F32, tag="rclB",
                                           bufs=1)
                            nc.vector.reciprocal(rclB, LstB[:, c, :])
                            emit_norm(c, range(4, 8), rclB)

                    def emit_proj(c, ems, act_po=False):
                        csl = ts(c, 512)
                        dst = partA if c % 2 == 0 else partB
                        slot = c // 2
                        for em in ems:
                            pP = ps.tile([128, 512], F32, tag="pS", bufs=2,
                                         name="pP")
                            for kh in range(4):
                                nc.tensor.matmul(
                                    pP, apws[:, kh, ts(em, 128)],
                                    AO[:, kh, csl],
                                    start=(kh == 0), stop=(kh == 3))
                            po = sb.tile([128, 512], BF16, tag="po", bufs=3)
                            if act_po:
                                nc.scalar.activation(po, pP, COPY)
                            else:
                                nc.vector.tensor_copy(po, pP)
                            nc.sync.dma_start(
                                out=dst[slot, ts(em, 128), :], in_=po)

                    def emit_rs(part, rsx):
                        if single:
                            nc.sync.dma_start(out=rsx[:], in_=part[0, :, :])
                        else:
                            nc.gpsimd.collective_compute(
                                "ReduceScatter", ADD, replica_groups=groups,
                                ins=[part[:]], outs=[rsx[:]])



                    ln2A = {}
                    for c in range(4):
                        for h in range(HPC):
                            emit_head(c, h)
                            # weave the previous column's projection in
                            # 2-E-tile slices between heads 1..4
                            if c in (1, 2) and 1 <= h <= 4:
                                emit_proj(c - 1, range(2 * (h - 1),
                                                       2 * h))
                            if c == 3 and h == 3:
                                # residual-A + LN2-A input prep overlap
                                # the rest of column 3 (rsA lands early)
                                emit_res_half(0, rsA, sb, xrs)
                                ln2A["t"] = emit_ln2_pre(0, sb,
                                                         on_act=False)
                        if c == 2:
                            # full proj(2) + early RS_A at column-2 end
                            emit_proj(2, range(ET))
                            emit_rs(partA, rsA)
                    # column 3 epilogue
                    emit_ln2_post(0, sb, ps, "pS", *ln2A["t"], h2A)
                    emit_proj(3, range(ET), act_po=True)
                    emit_rs(partB, rsB)
                    if dbg:
                        nc.sync.dma_start(out=dbg_t["dAO"], in_=AO)
                        nc.sync.dma_start(out=dbg_t["dLA"], in_=LstA)
                        nc.sync.dma_start(out=dbg_t["dLB"], in_=LstB)
                        nc.sync.dma_start(out=dbg_t["drsA"], in_=rsA[:])
                        nc.sync.dma_start(out=dbg_t["drsB"], in_=rsB[:])
                        nc.sync.dma_start(out=dbg_t["dQT"], in_=QT)
                        nc.sync.dma_start(out=dbg_t["dKT"], in_=KT)
                        nc.sync.dma_start(out=dbg_t["dVp"], in_=Vp)

            # ======== phase B: FFN (e-major), token-halved ========
            with tc.tile_pool(name="persB", bufs=1) as pB:
                nc.sync.dma_start(out=fb1_sb, in_=fb1)
                nc.sync.dma_start(out=fb2_sb, in_=fb2)
                if dbg:
                    nc.sync.dma_start(out=dbg_t["dxp"], in_=xp)
                h2B = pB.tile([128, ET, 512], BF16, name="h2B")
                ffh = pB.tile([128, FF // 128, TH], BF16, name="ffh")
                with tc.tile_pool(name="ffw", bufs=1) as sbw, \
                     tc.tile_pool(name="ff_ps", bufs=1, space="PSUM") as ps:
                    ln2B = {}
                    for half, h2 in ((0, h2A), (1, h2B)):
                        hsl = ds(512 * half, 512)
                        for m in range(FF // 128):
                            if half == 0 and m == 14:
                                xrsB = sbw.tile([128, ET, 512], BF16,
                                                name="xrsB")
                                nc.sync.dma_start(
                                    out=xrsB,
                                    in_=xrpT.rearrange(
                                        "(i p) t -> p i t",
                                        p=128)[:, :, ds(512, 512)])
                            if half == 0 and m == 16:
                                # residual-B once RS_B has landed
                                emit_res_half(1, rsB, sbw, xrsB)
                            if half == 0 and m == 20:
                                # LN2 for half B hides inside ff1(A)
                                ln2B["t"] = emit_ln2_pre(1, sbw)
                            if half == 0 and m == 24:
                                emit_ln2_post(1, sbw, ps, "pstat2",
                                              *ln2B["t"], h2B)
                            if m % 4 == 0:
                                w1e = sbw.tile([128, ET, 512], BF16,
                                               tag="w1e", bufs=2)
                                nc.sync.dma_start(
                                    out=w1e,
                                    in_=fw1.rearrange(
                                        "(i p) f -> p i f",
                                        p=128)[:, :, ds(512 * (m // 4), 512)])
                            pF = ps.tile([128, 512], F32, tag="pF", bufs=3)
                            for i in range(ET):
                                nc.tensor.matmul(
                                    pF, w1e[:, i, ts(m % 4, 128)],
                                    h2[:, i, :],
                                    start=(i == 0), stop=(i == 7))
                            nc.scalar.activation(
                                ffh[:, m, hsl], pF, RELU,
                                bias=fb1_sb[:, m:m + 1])
                    # ---- ff2 + residual + out, E-quarters x token halves ----
                    for eq in range(4):
                        w2q = sbw.tile([128, FF // 128, 256], BF16,
                                       tag="w2q", bufs=2)
                        nc.sync.dma_start(
                            out=w2q,
                            in_=fw2.rearrange("(k p) e -> p k e",
                                              p=128)[:, :,
                                                     ds(256 * eq, 256)])
                        for half in range(2):
                            hsl = ds(512 * half, 512)
                            for m2 in range(2):
                                m = 2 * eq + m2
                                pG = ps.tile([128, 512], F32, tag="pG",
                                             bufs=2)
                                for k in range(FF // 128):
                                    nc.tensor.matmul(
                                        pG, w2q[:, k, ts(m2, 128)],
                                        ffh[:, k, hsl],
                                        start=(k == 0), stop=(k == 31))
                                fin = sbw.tile([128, 512], F32, tag="fin",
                                               bufs=2)
                                nc.vector.tensor_tensor(
                                    fin, pG, xp[:, m, hsl], ADD)
                                nc.scalar.activation(
                                    fin, fin, IDENT,
                                    bias=fb2_sb[:, m:m + 1])
                                nc.sync.dma_start(
                                    out=out.rearrange(
                                        "(i p) t -> p i t", p=128)[:, m, hsl],
                                    in_=fin)

    with tile.TileContext(nc) as tc:
        _emit(tc)

    nc.compile()
    return nc


_CACHED = {}


def _prepare_inputs(x, qkv_w, attn_proj_w, attn_proj_b, ln1_g, ln1_b,
                    ln2_g, ln2_b, ff_w1, ff_b1, ff_w2, ff_b2):
    """Fold LN affine params into the weights, shard, and cast to bf16."""
    x = np.asarray(x, np.float32)
    qkv_w = np.asarray(qkv_w, np.float32) * np.asarray(ln1_g, np.float32)[:, None]
    qkv_b = np.asarray(ln1_b, np.float32) @ qkv_w  # [3*H*HS]
    assert np.abs(qkv_b).max() == 0.0, "nonzero ln1_b not supported"
    ff_w1f = np.asarray(ff_w1, np.float32) * np.asarray(ln2_g, np.float32)[:, None]
    ff_b1f = np.asarray(ff_b1, np.float32) + np.asarray(ln2_b, np.float32) @ ff_w1f
    apb = np.asarray(attn_proj_b, np.float32)

    fw1_bf = ff_w1f.astype(NPBF16)
    fw2_bf = np.asarray(ff_w2, np.float32).astype(NPBF16)
    fb1_t = np.ascontiguousarray(ff_b1f.reshape(FF // 128, 128).T)
    fb2_t = np.ascontiguousarray(
        np.asarray(ff_b2, np.float32).reshape(ET, 128).T)
    apw_bf = np.asarray(attn_proj_w, np.float32).astype(NPBF16)

    in_maps = []
    for c in range(NCORES):
        b, hh = c // 2, c % 2
        hsl = slice(512 * hh, 512 * hh + 512)
        tsl = slice(TH * hh, TH * hh + TH)
        in_maps.append({
            "xT": np.ascontiguousarray(x[b].T).astype(NPBF16),
            "xrpT": np.ascontiguousarray(
                (x[b, tsl] + apb[None, :]).T).astype(NPBF16),
            "qw": np.ascontiguousarray(qkv_w[:, hsl]).astype(NPBF16),
            "kw": np.ascontiguousarray(qkv_w[:, H * HS:][:, hsl]).astype(NPBF16),
            "vw": np.ascontiguousarray(qkv_w[:, 2 * H * HS:][:, hsl]).astype(NPBF16),
            "apw": np.ascontiguousarray(apw_bf[hsl, :]),
            "fw1": fw1_bf,
            "fb1": fb1_t,
            "fw2": fw2_bf,
            "fb2": fb2_t,
        })
    return in_maps


def kernel(**inputs):
    if "nc" not in _CACHED:
        _CACHED["nc"] = build_program()
    nc = _CACHED["nc"]
    in_maps = _prepare_inputs(**inputs)
    res = run_bass_kernel_spmd(nc, in_maps, list(range(NCORES)))
    full = np.empty((B, T, E), np.float32)
    for c in range(NCORES):
        b, hh = c // 2, c % 2
        full[b, TH * hh:TH * hh + TH] = res.results[c]["out"].T
    return full


# revision 28
# speedup vs baseline: 1.0142x; 1.0142x over previous
"""Fused pre-LN transformer block (causal MHA + FFN) on 8 TRN2 NeuronCores.

Sharding: core c handles batch b = c//2 and head-half hh = c%2 (8 of 16 heads).
Attention runs fully local per (batch, head-half); the attention projection
produces a partial sum that is ReduceScattered (over token dim) within each
core pair, so FFN runs token-sharded (1024 tokens/core). Output per core is
its token slice, stored E-major [E, TH]; the host transposes when gathering.

v4: per-engine streams are executed in emission order, so the program is
emitted as one interleaved schedule:
- LN1+QKV software-pipelined over 4 token chunks.
- Attention emitted column-outer (q-chunk) x head with exp groups of 3
  k-blocks; the psO (attn-V) matmuls for group g are emitted after the
  scores of group g+1, so the PE never sits behind the scalar engine's exp.
- Softmax denominators staged to 8 partitions; one batched reciprocal per
  4 heads; normalization applied lazily in-place on the raw attn output.
- The attention projection for column c is emitted in 2-E-tile slices
  between the heads of column c+1 (fills the column-boundary bubble);
  the two bf16 pair ReduceScatters are issued mid-column-3 / right after.
- LN2 for token-half A is emitted inside the attention region (overlaps
  proj(3)/RS_B); LN2 for half B is woven into the middle of ff1(A).
- FFN: fw1 streamed in 1MB eighths per half, fw2 in quarters; relu and
  biases fused on the scalar engine.
"""

import numpy as np
import ml_dtypes

import concourse.bass as bass
import concourse.mybir as mybir
import concourse.tile as tile
from concourse import bacc
from concourse.bass import ts, ds
from concourse.bass_utils import run_bass_kernel_spmd

BF16 = mybir.dt.bfloat16
F32 = mybir.dt.float32
NPBF16 = ml_dtypes.bfloat16

B, T, E = 4, 2048, 1024
H, HS = 16, 64
FF = 4 * E
EPS = 1e-5
NCORES = 8
HPC = 8            # heads per core
HD = HPC * HS      # 512 head dims per core
TH = T // 2        # 1024 tokens per core for FFN
KT_N = T // 128    # 16 k-tiles
ET = E // 128      # 8 e-tiles
ADD = mybir.AluOpType.add
SUB = mybir.AluOpType.subtract
MUL = mybir.AluOpType.mult
EXP = mybir.ActivationFunctionType.Exp
RELU = mybir.ActivationFunctionType.Relu
SQRT = mybir.ActivationFunctionType.Sqrt
COPY = mybir.ActivationFunctionType.Copy
IDENT = mybir.ActivationFunctionType.Identity


def build_program(single=False, dbg=False):
    nc = bacc.Bacc("TRN2", target_bir_lowering=False, debug=False,
                   num_devices=1 if single else NCORES)
    dbg_t = {}
    if dbg:
        for name, shape, dt in (
                ("dQT", [128, 4, T], BF16), ("dKT", [128, 4, T], BF16),
                ("dVp", [128, KT_N, HPC, 65], BF16),
                ("dAO", [128, 4, T], BF16), ("dLA", [97, 4, 512], F32),
                ("dLB", [97, 4, 512], F32), ("drsA", [E, 512], BF16),
                ("drsB", [E, 512], BF16), ("dxp", [128, ET, TH], F32)):
            dbg_t[name] = nc.dram_tensor(name, shape, dt,
                                         kind="ExternalOutput").ap()

    # ---- I/O ----
    xT = nc.dram_tensor("xT", [E, T], BF16, kind="ExternalInput").ap()
    xrpT = nc.dram_tensor("xrpT", [E, TH], BF16, kind="ExternalInput").ap()
    qw = nc.dram_tensor("qw", [E, HD], BF16, kind="ExternalInput").ap()
    kw = nc.dram_tensor("kw", [E, HD], BF16, kind="ExternalInput").ap()
    vw = nc.dram_tensor("vw", [E, HD], BF16, kind="ExternalInput").ap()
    apw = nc.dram_tensor("apw", [HD, E], BF16, kind="ExternalInput").ap()
    fw1 = nc.dram_tensor("fw1", [E, FF], BF16, kind="ExternalInput").ap()
    fb1 = nc.dram_tensor("fb1", [128, FF // 128], F32,
                         kind="ExternalInput").ap()
    fw2 = nc.dram_tensor("fw2", [FF, E], BF16, kind="ExternalInput").ap()
    fb2 = nc.dram_tensor("fb2", [128, ET], F32, kind="ExternalInput").ap()
    out = nc.dram_tensor("out", [E, TH], F32, kind="ExternalOutput").ap()

    # internal DRAM for the two chunked pair-wise reduce-scatters (bf16).
    # partA holds global token quarters {0, 2}, partB {1, 3}; RS over the
    # core pair scatters dim0, so the even core gets quarters 0,1 and the
    # odd core quarters 2,3 == its own token half.
    partA = nc.dram_tensor("partA", [2, E, 512], BF16).ap()
    partB = nc.dram_tensor("partB", [2, E, 512], BF16).ap()
    rsA = nc.dram_tensor("rsA", [E, 512], BF16).ap()
    rsB = nc.dram_tensor("rsB", [E, 512], BF16).ap()
    groups = [[0, 1], [2, 3], [4, 5], [6, 7]]

    def _emit(tc):
        with tc.tile_pool(name="const", bufs=1) as constp:
            ones_bf = constp.tile([128, 1], BF16)
            nc.gpsimd.memset(ones_bf, 1.0)
            eps_sb = constp.tile([1, 1], F32)
            nc.gpsimd.memset(eps_sb, EPS)
            scratch1 = constp.tile([1, 64], F32)
            nc.gpsimd.memset(scratch1, 1.0)
            fb1_sb = constp.tile([128, FF // 128], F32)
            fb2_sb = constp.tile([128, ET], F32)

            # x' (post-attention residual input), lives to the end.
            xp = constp.tile([128, ET, TH], F32, name="xp")
            # LN2 output for token half A (computed in the attn region)
            h2A = constp.tile([128, ET, 512], BF16, name="h2A")
            # first eighth of fw1, prefetched early so ff1(A) starts cold
            w1e0 = constp.tile([128, ET, 512], BF16, name="w1e0")

            def emit_ln2_pre(half, pool, on_act=True):
                hsl = ds(512 * half, 512)
                xpb = pool.tile([128, ET, 512], BF16, tag=f"xpb{half}",
                                bufs=1)
                if on_act:
                    nc.scalar.activation(xpb, xp[:, :, hsl], COPY)
                else:
                    nc.vector.tensor_copy(xpb, xp[:, :, hsl])
                xqb = pool.tile([128, ET, 512], BF16, tag=f"xqb{half}",
                                bufs=1)
                nc.vector.tensor_tensor(xqb, xpb, xpb, MUL)
                return xpb, xqb

            def emit_ln2_post(half, pool, psp, pstag, xpb, xqb, dst):
                psm = psp.tile([1, 512], F32, tag=pstag, bufs=2)
                psq = psp.tile([1, 512], F32, tag=pstag, bufs=2)
                for i in range(ET):
                    nc.tensor.matmul(psm, ones_bf, xpb[:, i, :],
                                     start=(i == 0), stop=(i == 7))
                for i in range(ET):
                    nc.tensor.matmul(psq, ones_bf, xqb[:, i, :],
                                     start=(i == 0), stop=(i == 7))
                mu = pool.tile([1, 512], F32, tag=f"mu_{half}", bufs=1)
                rstd = pool.tile([1, 512], F32, tag=f"rstd_{half}", bufs=1)
                msr = pool.tile([1, 512], F32, tag=f"msr_{half}", bufs=1)
                nc.vector.tensor_scalar_mul(mu, psm, 1.0 / E)
                nc.vector.tensor_scalar_mul(rstd, psq, 1.0 / E)
                nc.vector.tensor_tensor(msr, mu, mu, MUL)
                nc.vector.tensor_tensor(rstd, rstd, msr, SUB)
                nc.scalar.activation(rstd, rstd, SQRT, bias=eps_sb)
                nc.vector.reciprocal(rstd, rstd)
                nc.vector.tensor_tensor(msr, mu, rstd, MUL)
                bc_rs = pool.tile([128, 512], F32, tag=f"bcr_{half}",
                                  bufs=1)
                nc.gpsimd.partition_broadcast(bc_rs, rstd)
                bc_ms = pool.tile([128, 512], F32, tag=f"bcm_{half}",
                                  bufs=1)
                nc.gpsimd.partition_broadcast(bc_ms, msr)
                nc.vector.tensor_tensor(
                    dst, xpb,
                    bc_rs.unsqueeze(1).broadcast_to([128, ET, 512]), MUL)
                nc.vector.tensor_tensor(
                    dst, dst,
                    bc_ms.unsqueeze(1).broadcast_to([128, ET, 512]), SUB)

            def emit_res_half(half, rsx, pool, xsrc, eng=None):
                hsl = ds(512 * half, 512)
                rsb = pool.tile([128, ET, 512], BF16, tag=f"rsb{half}",
                                bufs=1)
                (eng or nc.sync).dma_start(
                    out=rsb, in_=rsx.rearrange("(j p) t -> p j t", p=128))
                nc.vector.tensor_tensor(xp[:, :, hsl], rsb, xsrc, ADD)

            # ======== phase A: attention ========
            with tc.tile_pool(name="persA", bufs=1) as pA:
                QT = pA.tile([128, 4, T], BF16, name="QT")
                KT = pA.tile([128, 4, T], BF16, name="KT")
                AO = pA.tile([128, 4, T], BF16, name="AO")
                Vp = pA.tile([128, KT_N, HPC, 65], BF16, name="Vp")
                nc.vector.memset(Vp[:, :, :, 64:65], 1.0)
                # softmax denominators: 4 heads per tile on partitions
                # {0,32,64,96} (DVE partition-start constraint), so the
                # per-column reciprocal batches 4 heads per op.
                LstA = pA.tile([97, 4, 512], F32, name="LstA")
                LstB = pA.tile([97, 4, 512], F32, name="LstB")
                nc.gpsimd.memset(LstA, 1.0)
                nc.gpsimd.memset(LstB, 1.0)
                # causal masks for the 4 diagonal k-offsets:
                # masks[p, kk, qq] = 1 if qq >= 128*kk + p else 0
                masks = pA.tile([128, 4, 512], BF16, name="masks")
                nc.gpsimd.memset(masks, 1.0)
                nc.gpsimd.affine_select(
                    out=masks, in_=masks, compare_op=mybir.AluOpType.is_ge,
                    fill=0.0, base=0, pattern=[[-128, 4], [1, 512]],
                    channel_multiplier=-1)

                # ---- A1: LN1 + QKV, software-pipelined over 4 chunks ----
                with tc.tile_pool(name="ln1", bufs=1) as sb, \
                     tc.tile_pool(name="ln1_ps", bufs=1, space="PSUM") as ps:
                    xTs = sb.tile([128, ET, T], BF16, name="xTs")
                    w_q = sb.tile([128, ET, HD], BF16, name="w_q")
                    w_k = sb.tile([128, ET, HD], BF16, name="w_k")
                    w_v = sb.tile([128, ET, HD], BF16, name="w_v")
                    nc.sync.dma_start(
                        out=xTs[:, :, ds(0, 512)],
                        in_=xT.rearrange("(i p) t -> p i t",
                                         p=128)[:, :, ds(0, 512)])
                    nc.sync.dma_start(
                        out=w_q, in_=qw.rearrange("(i p) f -> p i f", p=128))
                    nc.sync.dma_start(
                        out=w_k, in_=kw.rearrange("(i p) f -> p i f", p=128))
                    nc.sync.dma_start(
                        out=w_v, in_=vw.rearrange("(i p) f -> p i f", p=128))

                    def emit_stats(c):
                        csl = ds(512 * c, 512)
                        if c > 0:
                            nc.sync.dma_start(
                                out=xTs[:, :, csl],
                                in_=xT.rearrange("(i p) t -> p i t",
                                                 p=128)[:, :, csl])
                        xsq = sb.tile([128, ET, 512], BF16, tag="xsq",
                                      bufs=1)
                        nc.vector.tensor_tensor(
                            xsq, xTs[:, :, csl], xTs[:, :, csl], MUL)
                        psm = ps.tile([1, 512], F32, tag="psm", bufs=3)
                        psq = ps.tile([1, 512], F32, tag="psq", bufs=3)
                        for i in range(ET):
                            nc.tensor.matmul(
                                psm, ones_bf, xTs[:, i, csl],
                                start=(i == 0), stop=(i == 7))
                        for i in range(ET):
                            nc.tensor.matmul(
                                psq, ones_bf, xsq[:, i, :],
                                start=(i == 0), stop=(i == 7))
                        mu = sb.tile([1, 512], F32, tag="mu", bufs=1)
                        rstd = sb.tile([1, 512], F32, tag="rstd", bufs=1)
                        msr = sb.tile([1, 512], F32, tag="msr", bufs=1)
                        nc.vector.tensor_scalar_mul(mu, psm, 1.0 / E)
                        nc.vector.tensor_scalar_mul(rstd, psq, 1.0 / E)
                        nc.vector.tensor_tensor(msr, mu, mu, MUL)
                        nc.vector.tensor_tensor(rstd, rstd, msr, SUB)
                        # rstd = 1/sqrt(var + eps)
                        nc.scalar.activation(rstd, rstd, SQRT, bias=eps_sb)
                        nc.vector.reciprocal(rstd, rstd)
                        nc.vector.tensor_tensor(msr, mu, rstd, MUL)
                        bc_rs = sb.tile([128, 512], F32, tag="bc_rs", bufs=1)
                        nc.gpsimd.partition_broadcast(bc_rs, rstd)
                        bc_ms = sb.tile([128, 512], F32, tag="bc_ms", bufs=1)
                        nc.gpsimd.partition_broadcast(bc_ms, msr)
                        # normalize in place: h = x*rstd - mu*rstd
                        nc.vector.tensor_tensor(
                            xTs[:, :, csl], xTs[:, :, csl],
                            bc_rs.unsqueeze(1).broadcast_to([128, ET, 512]),
                            MUL)
                        nc.vector.tensor_tensor(
                            xTs[:, :, csl], xTs[:, :, csl],
                            bc_ms.unsqueeze(1).broadcast_to([128, ET, 512]),
                            SUB)

                    def emit_qkv(c):
                        csl = ds(512 * c, 512)
                        for wi, (w_sb, o_sb) in enumerate(
                                ((w_q, QT), (w_k, KT))):
                            for m in range(4):
                                pq = ps.tile([128, 512], F32, tag="pqv",
                                             bufs=2)
                                for i in range(ET):
                                    nc.tensor.matmul(
                                        pq, w_sb[:, i, ts(m, 128)],
                                        xTs[:, i, csl],
                                        start=(i == 0), stop=(i == 7))
                                if (wi * 4 + m) % 2 == 0:
                                    nc.vector.tensor_copy(
                                        o_sb[:, m, csl], pq)
                                else:
                                    nc.scalar.activation(
                                        o_sb[:, m, csl], pq, COPY)
                        for mt in range(4):
                            kt = 4 * c + mt
                            pv = ps.tile([128, 512], F32, tag="pqv", bufs=2)
                            for i in range(ET):
                                nc.tensor.matmul(
                                    pv, xTs[:, i, ts(kt, 128)], w_v[:, i, :],
                                    start=(i == 0), stop=(i == 7))
                            vdst = Vp[:, kt, :, 0:64]
                            pvr = pv.rearrange("p (h d) -> p h d", h=HPC)
                            if mt % 2 == 0:
                                nc.vector.tensor_copy(vdst, pvr)
                            else:
                                nc.scalar.activation(vdst, pvr, COPY)

                    emit_stats(0)
                    emit_stats(1)
                    emit_stats(2)
                    emit_qkv(0)
                    emit_stats(3)
                    emit_qkv(1)
                    emit_qkv(2)
                    emit_qkv(3)
                    # preload the exp table while QKV finishes
                    nc.scalar.activation(scratch1, scratch1, EXP)

                # ---- A2+A3: attention/proj/RS/LN2-A interleaved ----
                with tc.tile_pool(name="att", bufs=1) as sb, \
                     tc.tile_pool(name="att_ps", bufs=1,
                                  space="PSUM") as ps:
                    apws = sb.tile([128, 4, E], BF16, name="apws")
                    nc.sync.dma_start(
                        out=apws, in_=apw.rearrange("(k p) e -> p k e",
                                                    p=128))
                    nc.sync.dma_start(
                        out=w1e0, in_=fw1.rearrange(
                            "(i p) f -> p i f", p=128)[:, :, ds(0, 512)])
                    xrs = sb.tile([128, ET, 512], BF16, name="xrs")
                    nc.sync.dma_start(
                        out=xrs,
                        in_=xrpT.rearrange("(i p) t -> p i t",
                                           p=128)[:, :, ds(0, 512)])
                    G = 3

                    def emit_norm(c, heads, rcl):
                        csl = ts(c, 512)
                        for h in heads:
                            hp, z = h // 2, h % 2
                            pp = slice(64 * z, 64 * z + 64)
                            lp = 32 * (h % 4)
                            # HW partition_broadcast reads the tile's
                            # partition 0 only -> bounce through a p0 row.
                            t0 = sb.tile([1, 512], F32, tag="t0", bufs=2)
                            nc.vector.tensor_copy(t0, rcl[lp:lp + 1, :])
                            rbc = sb.tile([128, 512], F32, tag="rbc",
                                          bufs=2)
                            nc.gpsimd.partition_broadcast(rbc, t0)
                            nc.vector.tensor_tensor(
                                AO[pp, hp, csl], AO[pp, hp, csl],
                                rbc[pp, :], MUL)

                    def emit_head(c, h):
                        csl = ts(c, 512)
                        njs = 4 * (c + 1)
                        hp, z = h // 2, h % 2
                        pp = slice(64 * z, 64 * z + 64)
                        psO = ps.tile([65, 512], F32, tag="psO", bufs=2,
                                      name="psO")

                        def psO_mm(js, PT):
                            for idx, j in enumerate(js):
                                nc.tensor.matmul(
                                    psO, Vp[:, j, h, :], PT[:, idx, :],
                                    start=(j == 0), stop=(j == njs - 1))

                        pend = []
                        for g0 in range(0, njs, G):
                            js = list(range(g0, min(g0 + G, njs)))
                            n = len(js)
                            pS = ps.tile([128, G, 512], F32, tag="pS",
                                         bufs=2, name="pS")
                            for idx, j in enumerate(js):
                                nc.tensor.matmul(
                                    pS[:, idx, :], KT[pp, hp, ts(j, 128)],
                                    QT[pp, hp, csl], start=True, stop=True)
                            PT = sb.tile([128, G, 512], BF16, tag="PT",
                                         bufs=3, name="PT")
                            nc.scalar.activation(
                                PT[:, 0:n, :], pS[:, 0:n, :], EXP,
                                scale=float(HS) ** -0.5)
                            d0 = 4 * c
                            if js[-1] >= d0:
                                lo = max(js[0], d0)
                                a = lo - js[0]
                                nc.vector.tensor_tensor(
                                    PT[:, a:n, :], PT[:, a:n, :],
                                    masks[:, lo - d0:js[-1] - d0 + 1, :],
                                    MUL)
                            pend.append((js, PT))
                            if len(pend) > 2:
                                psO_mm(*pend.pop(0))
                        for e in pend:
                            psO_mm(*e)
                        # stage raw output + denominator; normalize later
                        nc.vector.tensor_copy(AO[pp, hp, csl], psO[0:64, :])
                        lst = LstA if h < 4 else LstB
                        lp = 32 * (h % 4)
                        nc.vector.tensor_copy(lst[lp:lp + 1, c, :],
                                              psO[64:65, :])
                        if h == 3:
                            # heads 0-3 normalize overlaps heads 4-7
                            rclA = sb.tile([97, 512], F32, tag="rclA",
                                           bufs=1)
                            nc.vector.reciprocal(rclA, LstA[:, c, :])
                            emit_norm(c, range(0, 4), rclA)
                        if h == 7:
                            rclB = sb.tile([97, 512], F32, tag="rclB",
                                           bufs=1)
                            nc.vector.reciprocal(rclB, LstB[:, c, :])
                            emit_norm(c, range(4, 8), rclB)

                    def emit_proj(c, ems, act_po=False):
                        csl = ts(c, 512)
                        dst = partA if c % 2 == 0 else partB
                        slot = c // 2
                        for em in ems:
                            pP = ps.tile([128, 512], F32, tag="pS", bufs=2,
                                         name="pP")
                            for kh in range(4):
                                nc.tensor.matmul(
                                    pP, apws[:, kh, ts(em, 128)],
                                    AO[:, kh, csl],
                                    start=(kh == 0), stop=(kh == 3))
                            po = sb.tile([128, 512], BF16, tag="po", bufs=2)
                            if act_po:
                                nc.scalar.activation(po, pP, COPY)
                            else:
                                nc.vector.tensor_copy(po, pP)
                            nc.sync.dma_start(
                                out=dst[slot, ts(em, 128), :], in_=po)

                    def emit_rs(part, rsx):
                        if single:
                            nc.sync.dma_start(out=rsx[:], in_=part[0, :, :])
                        else:
                            nc.gpsimd.collective_compute(
                                "ReduceScatter", ADD, replica_groups=groups,
                                ins=[part[:]], outs=[rsx[:]])



                    ln2A = {}
                    for c in range(4):
                        for h in range(HPC):
                            emit_head(c, h)
                            # weave the previous column's projection in
                            # 2-E-tile slices between heads 1..4
                            if c in (1, 2) and 1 <= h <= 4:
                                emit_proj(c - 1, range(2 * (h - 1),
                                                       2 * h))
                            if c == 3 and h == 6:
                                # residual-A + LN2-A input prep; placed
                                # late enough that rsA has landed
                                emit_res_half(0, rsA, sb, xrs)
                                ln2A["t"] = emit_ln2_pre(0, sb,
                                                         on_act=False)
                        if c == 2:
                            # full proj(2) + early RS_A at column-2 end
                            emit_proj(2, range(ET))
                            emit_rs(partA, rsA)
                    # column 3 epilogue
                    emit_ln2_post(0, sb, ps, "pS", *ln2A["t"], h2A)
                    emit_proj(3, range(ET), act_po=True)
                    emit_rs(partB, rsB)
                    if dbg:
                        nc.sync.dma_start(out=dbg_t["dAO"], in_=AO)
                        nc.sync.dma_start(out=dbg_t["dLA"], in_=LstA)
                        nc.sync.dma_start(out=dbg_t["dLB"], in_=LstB)
                        nc.sync.dma_start(out=dbg_t["drsA"], in_=rsA[:])
                        nc.sync.dma_start(out=dbg_t["drsB"], in_=rsB[:])
                        nc.sync.dma_start(out=dbg_t["dQT"], in_=QT)
                        nc.sync.dma_start(out=dbg_t["dKT"], in_=KT)
                        nc.sync.dma_start(out=dbg_t["dVp"], in_=Vp)

            # ======== phase B: FFN (e-major), token-halved ========
            with tc.tile_pool(name="persB", bufs=1) as pB:
                nc.sync.dma_start(out=fb1_sb, in_=fb1)
                nc.sync.dma_start(out=fb2_sb, in_=fb2)
                if dbg:
                    nc.sync.dma_start(out=dbg_t["dxp"], in_=xp)
                h2B = pB.tile([128, ET, 512], BF16, name="h2B")
                ffh = pB.tile([128, FF // 128, TH], BF16, name="ffh")
                with tc.tile_pool(name="ffw", bufs=1) as sbw, \
                     tc.tile_pool(name="ff_ps", bufs=1, space="PSUM") as ps:
                    ln2B = {}
                    for half, h2 in ((0, h2A), (1, h2B)):
                        hsl = ds(512 * half, 512)
                        for m in range(FF // 128):
                            if half == 0 and m == 14:
                                xrsB = sbw.tile([128, ET, 512], BF16,
                                                name="xrsB")
                                nc.gpsimd.dma_start(
                                    out=xrsB,
                                    in_=xrpT.rearrange(
                                        "(i p) t -> p i t",
                                        p=128)[:, :, ds(512, 512)])
                            if half == 0 and m == 16:
                                # residual-B once RS_B has landed; the DMA
                                # goes on the idle gpsimd SWDGE ring so the
                                # RS_B wait cannot block the fw1 fetches
                                emit_res_half(1, rsB, sbw, xrsB,
                                              eng=nc.gpsimd)
                            if half == 0 and m == 20:
                                # LN2 for half B hides inside ff1(A)
                                ln2B["t"] = emit_ln2_pre(1, sbw)
                            if half == 0 and m == 24:
                                emit_ln2_post(1, sbw, ps, "pstat2",
                                              *ln2B["t"], h2B)
                            if m % 4 == 0 and (half, m) == (0, 0):
                                w1e = w1e0
                            elif m % 4 == 0:
                                w1e = sbw.tile([128, ET, 512], BF16,
                                               tag="w1e", bufs=2)
                                nc.sync.dma_start(
                                    out=w1e,
                                    in_=fw1.rearrange(
                                        "(i p) f -> p i f",
                                        p=128)[:, :, ds(512 * (m // 4), 512)])
                            pF = ps.tile([128, 512], F32, tag="pF", bufs=3)
                            for i in range(ET):
                                nc.tensor.matmul(
                                    pF, w1e[:, i, ts(m % 4, 128)],
                                    h2[:, i, :],
                                    start=(i == 0), stop=(i == 7))
                            nc.scalar.activation(
                                ffh[:, m, hsl], pF, RELU,
                                bias=fb1_sb[:, m:m + 1])
                    # ---- ff2 + residual + out, E-eighths x token halves ----
                    for m in range(ET):
                        w2q = sbw.tile([128, FF // 128, 128], BF16,
                                       tag="w2q", bufs=2)
                        nc.sync.dma_start(
                            out=w2q,
                            in_=fw2.rearrange("(k p) e -> p k e",
                                              p=128)[:, :,
                                                     ds(128 * m, 128)])
                        for half in range(2):
                            hsl = ds(512 * half, 512)
                            if True:
                                pG = ps.tile([128, 512], F32, tag="pG",
                                             bufs=2)
                                for k in range(FF // 128):
                                    nc.tensor.matmul(
                                        pG, w2q[:, k, :],
                                        ffh[:, k, hsl],
                                        start=(k == 0), stop=(k == 31))
                                fin = sbw.tile([128, 512], F32, tag="fin",
                                               bufs=2)
                                nc.vector.tensor_tensor(
                                    fin, pG, xp[:, m, hsl], ADD)
                                nc.scalar.activation(
                                    fin, fin, IDENT,
                                    bias=fb2_sb[:, m:m + 1])
                                nc.sync.dma_start(
                                    out=out.rearrange(
                                        "(i p) t -> p i t", p=128)[:, m, hsl],
                                    in_=fin)

    with tile.TileContext(nc) as tc:
        _emit(tc)

    nc.compile()
    return nc


_CACHED = {}


def _prepare_inputs(x, qkv_w, attn_proj_w, attn_proj_b, ln1_g, ln1_b,
                    ln2_g, ln2_b, ff_w1, ff_b1, ff_w2, ff_b2):
    """Fold LN affine params into the weights, shard, and cast to bf16."""
    x = np.asarray(x, np.float32)
    qkv_w = np.asarray(qkv_w, np.float32) * np.asarray(ln1_g, np.float32)[:, None]
    qkv_b = np.asarray(ln1_b, np.float32) @ qkv_w  # [3*H*HS]
    assert np.abs(qkv_b).max() == 0.0, "nonzero ln1_b not supported"
    ff_w1f = np.asarray(ff_w1, np.float32) * np.asarray(ln2_g, np.float32)[:, None]
    ff_b1f = np.asarray(ff_b1, np.float32) + np.asarray(ln2_b, np.float32) @ ff_w1f
    apb = np.asarray(attn_proj_b, np.float32)

    fw1_bf = ff_w1f.astype(NPBF16)
    fw2_bf = np.asarray(ff_w2, np.float32).astype(NPBF16)
    fb1_t = np.ascontiguousarray(ff_b1f.reshape(FF // 128, 128).T)
    fb2_t = np.ascontiguousarray(
        np.asarray(ff_b2, np.float32).reshape(ET, 128).T)
    apw_bf = np.asarray(attn_proj_w, np.float32).astype(NPBF16)

    in_maps = []
    for c in range(NCORES):
        b, hh = c // 2, c % 2
        hsl = slice(512 * hh, 512 * hh + 512)
        tsl = slice(TH * hh, TH * hh + TH)
        in_maps.append({
            "xT": np.ascontiguousarray(x[b].T).astype(NPBF16),
            "xrpT": np.ascontiguousarray(
                (x[b, tsl] + apb[None, :]).T).astype(NPBF16),
            "qw": np.ascontiguousarray(qkv_w[:, hsl]).astype(NPBF16),
            "kw": np.ascontiguousarray(qkv_w[:, H * HS:][:, hsl]).astype(NPBF16),
            "vw": np.ascontiguousarray(qkv_w[:, 2 * H * HS:][:, hsl]).astype(NPBF16),
            "apw": np.ascontiguousarray(apw_bf[hsl, :]),
            "fw1": fw1_bf,
            "fb1": fb1_t,
            "fw2": fw2_bf,
            "fb2": fb2_t,
        })
    return in_maps


def kernel(**inputs):
    if "nc" not in _CACHED:
        _CACHED["nc"] = build_program()
    nc = _CACHED["nc"]
    in_maps = _prepare_inputs(**inputs)
    res = run_bass_kernel_spmd(nc, in_maps, list(range(NCORES)))
    full = np.empty((B, T, E), np.float32)
    for c in range(NCORES):
        b, hh = c // 2, c % 2
        full[b, TH * hh:TH * hh + TH] = res.results[c]["out"].T
    return full
